# revision 19
# baseline (speedup 1.0000x reference)
"""Trainium2 Bass kernel: attention with LayerNorm on scores (sparse_attention).

Per-core work (1 of 8 heads, data-parallel over batch axis n):
    S   = (Q @ K^T) * 1/sqrt(d)          [L, L]
    Sn  = LayerNorm(S, axis=-1) * gamma + beta
    A   = softmax(Sn, axis=-1)
    out = A @ V                           [L, D]

Fast path (gamma==1, beta==0), S-transposed design:
  - softmax is invariant to per-row shifts, so the LN mean term and beta
    cancel; only the per-row scale a_q = scale*rstd_q survives. It is
    folded into Q before the QK matmul (per-partition multiply in natural
    layout), so S^T tiles go straight into a bias/scale-free ACT exp.
  - S^T = K @ (aQ)^T is computed with k on PSUM partitions, so exp output
    is already key-major for the PV matmul: no attn transpose at all.
  - LN stats come algebraically from G = K^T[K|1]: H = QG gives per-query
    sum-of-squares via rowsum(H o Q) and the mean via the ksum column.
  - All matmuls run in fp16 (fp32/fp32r matmuls are 2-pass on TRN2 HW).
  - softmax denominators: DVE accumulates exp tiles elementwise in fp16,
    one thin ones-matmul reduces over partitions, and a DRAM bounce +
    PE transpose lands 1/den in per-partition layout for the final scale.
  - out^T from PV is copied fp16 and DMA-xbar-transposed back to natural
    layout; the 1/den scale rides the final PSUM->SBUF per-tile copy.
"""

import numpy as np
from contextlib import ExitStack

import concourse.bass as bass
import concourse.bacc as bacc
import concourse.tile as tile
from concourse import mybir
from concourse import bass_utils
from concourse.masks import make_identity

F32 = mybir.dt.float32
F32R = mybir.dt.float32r
BF16 = mybir.dt.bfloat16
FP16 = mybir.dt.float16
AF = mybir.ActivationFunctionType
ALU = mybir.AluOpType
AX = mybir.AxisListType

EPS = 1e-5
N_CORES = 8


def build_kernel_fast(L=2048, D=128):
    """Trivial-affine (gamma=1, beta=0) fast kernel."""
    P = 128
    T = L // P          # 16 row tiles
    HQ = L // 2         # q-half width (1024)
    HT = T // 2         # q tiles per half (8)
    scale = 1.0 / np.sqrt(np.float32(D))

    nc = bacc.Bacc(
        "TRN2",
        target_bir_lowering=False,
        debug=False,
        enable_asserts=False,
        num_devices=N_CORES,
    )
    q_d = nc.dram_tensor("q", [L, D], F32, kind="ExternalInput").ap()
    k_d = nc.dram_tensor("k", [L, D], F32, kind="ExternalInput").ap()
    v_d = nc.dram_tensor("v", [L, D], F32, kind="ExternalInput").ap()
    out_d = nc.dram_tensor("out", [L, D], F32, kind="ExternalOutput").ap()

    with tile.TileContext(nc) as tc, ExitStack() as ctx:
        consts = ctx.enter_context(tc.tile_pool(name="consts", bufs=1))

        ident16 = consts.tile([P, P], FP16)
        eps2_t = consts.tile([P, 1], F32)

        q_sb = consts.tile([P, T, D], F32)
        k_sb = consts.tile([P, T, D], F32)
        v_sb = consts.tile([P, T, D], F32)
        k16 = consts.tile([P, T, D + 1], FP16)   # K + ones col (for G)
        q16 = consts.tile([P, T, D], FP16)
        v16 = consts.tile([P, T, D], FP16)
        qs16 = consts.tile([P, T, D], FP16)      # a_q-scaled Q
        qT16 = consts.tile([P, L], FP16)         # Q^T (d-major, for H)
        kT16 = consts.tile([P, L], FP16)         # K^T (S^T stationary)
        qsT16 = consts.tile([P, L], FP16)        # (aQ)^T (S^T rhs)
        G16 = consts.tile([P, D + 1], FP16)

        junk = consts.tile([P, HT, D], F32)
        esq = consts.tile([P, T, 1], F32)
        nmu = consts.tile([P, T, 1], F32)
        var = consts.tile([P, T, 1], F32)
        rstd = consts.tile([P, T, 1], F32)
        rden3 = consts.tile([P, T, 1], F32)

        denacc = consts.tile([P, L], FP16)       # per-(k-row, q) exp sums
        o16 = consts.tile([P, L], FP16)          # out^T (d-major)
        out16 = consts.tile([P, T, D], FP16)     # natural out, fp16
        out_sb = consts.tile([P, T, D], F32)     # final scaled out

        # ---- loads first; Q is chain-critical so it gets queue priority -
        def load_half(eng, dst, src, c):
            eng.dma_start(
                out=dst[:, c * HT : (c + 1) * HT, :],
                in_=src[c * HQ : (c + 1) * HQ, :].rearrange(
                    "(t p) d -> p t d", p=P
                ),
            )

        load_half(nc.sync, q_sb, q_d, 0)
        load_half(nc.gpsimd, q_sb, q_d, 1)
        load_half(nc.scalar, k_sb, k_d, 0)
        load_half(nc.sync, k_sb, k_d, 1)
        load_half(nc.scalar, v_sb, v_d, 0)
        load_half(nc.scalar, v_sb, v_d, 1)

        make_identity(nc, ident16)
        nc.vector.memset(eps2_t, EPS * D)
        nc.vector.memset(k16[:, :, D : D + 1], 1.0)

        # fp16 casts on DVE, in expected arrival order
        def cast_half(dst, src, c, lastcol=D + 1):
            nc.vector.tensor_copy(
                out=dst[:, c * HT : (c + 1) * HT, 0:D],
                in_=src[:, c * HT : (c + 1) * HT, :],
            )

        cast_half(q16, q_sb, 0)
        cast_half(k16, k_sb, 0)
        cast_half(q16, q_sb, 1)
        cast_half(k16, k_sb, 1)

        # ---- precompute: transposes, G, H, stats, scaled Q --------------
        with tc.tile_pool(name="pre_ps", bufs=2, space="PSUM") as pre_ps:
            # Q^T via fp16 transposes first (G waits on the K tail)
            for b in range(4):
                trp = pre_ps.tile([P, 512], FP16, tag="tr", name=f"qtr_{b}")
                for j in range(4):
                    t = 4 * b + j
                    nc.tensor.transpose(
                        trp[:, j * P : (j + 1) * P], q16[:, t, :], ident16
                    )
                nc.scalar.copy(out=qT16[:, b * 512 : (b + 1) * 512], in_=trp)

            # G' = sum_t K_t^T [K_t | 1]  -> [D, D+1]  (rides the tr ring)
            g_ps = pre_ps.tile([P, D + 1], F32, tag="tr", name="g_ps")
            for t in range(T):
                nc.tensor.matmul(
                    g_ps,
                    lhsT=k16[:, t, 0:D],
                    rhs=k16[:, t, :],
                    start=(t == 0),
                    stop=(t == T - 1),
                )
            nc.scalar.copy(out=G16, in_=g_ps)

            def stats_batch(b):
                """H_b = Q_b G' (+ksum col); sigma-scaled moments so that
                rstd' = 1/sqrt(var' + eps*D) equals scale*rstd directly."""
                h_ps = pre_ps.tile([P, HT, D], F32, tag="h", name=f"h_{b}")
                hm_ps = pre_ps.tile([P, HT, 1], F32, tag="hm", name=f"hm_{b}")
                for j in range(HT):
                    t = HT * b + j
                    nc.tensor.matmul(
                        h_ps[:, j, :],
                        lhsT=qT16[:, t * P : (t + 1) * P],
                        rhs=G16[:, 0:D],
                        start=True,
                        stop=True,
                    )
                    nc.tensor.matmul(
                        hm_ps[:, j, :],
                        lhsT=qT16[:, t * P : (t + 1) * P],
                        rhs=G16[:, D : D + 1],
                        start=True,
                        stop=True,
                    )
                s8 = slice(HT * b, HT * b + HT)
                nc.vector.tensor_tensor(
                    junk, h_ps, q_sb[:, s8, :], op=ALU.mult
                )
                nc.vector.tensor_reduce(
                    esq[:, s8, :], junk, axis=AX.X, op=ALU.add
                )
                nc.vector.tensor_scalar_mul(nmu[:, s8, :], hm_ps, -1.0 / L)

            stats_batch(0)
            stats_batch(1)
            nc.vector.tensor_scalar_mul(esq, esq, 1.0 / L)
            nc.vector.tensor_tensor(var, nmu, nmu, op=ALU.mult)
            nc.vector.tensor_sub(var, esq, var)
            nc.scalar.activation(rstd, var, AF.Sqrt, bias=eps2_t)
            nc.vector.reciprocal(rstd, rstd)

            # K^T transposes fill the PE while DVE runs stats
            for b in range(4):
                trp = pre_ps.tile([P, 512], FP16, tag="tr", name=f"ktr_{b}")
                for j in range(4):
                    t = 4 * b + j
                    nc.tensor.transpose(
                        trp[:, j * P : (j + 1) * P], k16[:, t, 0:D], ident16
                    )
                if b == 0:
                    nc.scalar.copy(
                        out=kT16[:, b * 512 : (b + 1) * 512], in_=trp
                    )
                else:
                    nc.vector.tensor_copy(
                        out=kT16[:, b * 512 : (b + 1) * 512], in_=trp
                    )

            # scaled Q: one broadcast multiply per half, then transposes
            for c in range(2):
                hts = slice(c * HT, (c + 1) * HT)
                nc.vector.tensor_tensor(
                    qs16[:, hts, :],
                    q16[:, hts, :],
                    rstd[:, hts, :].to_broadcast((P, HT, D)),
                    op=ALU.mult,
                )
                for b in (2 * c, 2 * c + 1):
                    trp = pre_ps.tile(
                        [P, 512], FP16, tag="tr", name=f"qstr_{b}"
                    )
                    for j in range(4):
                        t = 4 * b + j
                        nc.tensor.transpose(
                            trp[:, j * P : (j + 1) * P],
                            qs16[:, t, :],
                            ident16,
                        )
                    if c == 0:
                        nc.scalar.copy(
                            out=qsT16[:, b * 512 : (b + 1) * 512], in_=trp
                        )
                    else:
                        nc.vector.tensor_copy(
                            out=qsT16[:, b * 512 : (b + 1) * 512], in_=trp
                        )

            # V -> fp16 per tile on gpsimd (needed from first PV onward)
            for t in range(T):
                nc.gpsimd.tensor_copy(out=v16[:, t, :], in_=v_sb[:, t, :])

        # ---- main: S^T -> exp -> PV, two q-halves, 8 PSUM banks ---------
        with (
            tc.tile_pool(name="mm_ps", bufs=2, space="PSUM") as mm_ps,
            tc.tile_pool(name="attn", bufs=3) as attn_pool,
        ):
            def emit_s(h, kt):
                s_ps = mm_ps.tile([P, HQ], F32, tag="s", name=f"s_{h}_{kt}")
                for c in range(2):
                    nc.tensor.matmul(
                        s_ps[:, c * 512 : (c + 1) * 512],
                        lhsT=kT16[:, kt * P : (kt + 1) * P],
                        rhs=qsT16[:, h * HQ + c * 512 : h * HQ + (c + 1) * 512],
                        start=True,
                        stop=True,
                    )
                return s_ps

            def post_half(h):
                """Drain one half in pipelined quarter-chunks: out^T copy,
                xbar transpose back to natural layout, transpose-based
                softmax denominators, broadcast 1/den scale, store."""
                hs = slice(h * HQ, (h + 1) * HQ)
                steps = []

                def mk_ocopy(c):
                    def p_ocopy():
                        cs = slice(h * HQ + c * 512, h * HQ + (c + 1) * 512)
                        if h == 0:
                            nc.vector.tensor_copy(
                                out=o16[:, cs], in_=o_ps[h][:, c * 512 : (c + 1) * 512]
                            )
                        else:
                            nc.scalar.copy(
                                out=o16[:, cs], in_=o_ps[h][:, c * 512 : (c + 1) * 512]
                            )
                    return p_ocopy

                def mk_xbar(c):
                    def p_xbar():
                        qts = slice(h * HT + 4 * c, h * HT + 4 * (c + 1))
                        cs = slice(h * HQ + c * 512, h * HQ + (c + 1) * 512)
                        nc.sync.dma_start_transpose(out16[:, qts, :], o16[:, cs])
                    return p_xbar

                def p_dentr():
                    den_ps[h] = mm_ps.tile(
                        [P, HT, P], FP16, tag="s", name=f"dentr_{h}"
                    )
                    for j in range(HT):
                        t = h * HT + j
                        nc.tensor.transpose(
                            den_ps[h][:, j, :],
                            denacc[:, t * P : (t + 1) * P],
                            ident16,
                        )

                def p_denred():
                    hts = slice(h * HT, (h + 1) * HT)
                    nc.vector.tensor_reduce(
                        rden3[:, hts, :], den_ps[h], axis=AX.X, op=ALU.add
                    )
                    nc.vector.reciprocal(rden3[:, hts, :], rden3[:, hts, :])

                def mk_scale_store(c):
                    def p_ss():
                        qts = slice(h * HT + 4 * c, h * HT + 4 * (c + 1))
                        nc.vector.tensor_tensor(
                            out_sb[:, qts, :],
                            out16[:, qts, :],
                            rden3[:, qts, :].to_broadcast((P, 4, D)),
                            op=ALU.mult,
                        )
                        nc.sync.dma_start(
                            out=out_d.rearrange("(t p) d -> p t d", p=P)[
                                :, qts, :
                            ],
                            in_=out_sb[:, qts, :],
                        )
                    return p_ss

                steps = [
                    mk_ocopy(0), p_dentr, mk_xbar(0), p_denred,
                    mk_ocopy(1), mk_xbar(1),
                    mk_scale_store(0), mk_scale_store(1),
                ]
                return steps

            o_ps = {}
            den_ps = {}
            pending = []         # post-half-0 steps drip-fed into half 1
            s_cur = emit_s(0, 0)
            for h in range(2):
                hs = slice(h * HQ, (h + 1) * HQ)
                o_ps[h] = mm_ps.tile([P, HQ], F32, tag="o", name=f"o_{h}")
                for kt in range(T):
                    attnT = attn_pool.tile(
                        [P, HQ], FP16, tag="attnT", name=f"attnT_{h}_{kt}"
                    )
                    nc.scalar.activation(attnT, s_cur, AF.Exp)
                    if kt == 0:
                        nc.vector.tensor_copy(out=denacc[:, hs], in_=attnT)
                    else:
                        nc.vector.tensor_tensor(
                            denacc[:, hs], denacc[:, hs], attnT, op=ALU.add
                        )
                    if (h, kt) != (1, T - 1):
                        nh, nkt = (h, kt + 1) if kt + 1 < T else (h + 1, 0)
                        s_cur = emit_s(nh, nkt)
                    for c in range(2):
                        nc.tensor.matmul(
                            o_ps[h][:, c * 512 : (c + 1) * 512],
                            lhsT=v16[:, kt, :],
                            rhs=attnT[:, c * 512 : (c + 1) * 512],
                            start=(kt == 0),
                            stop=(kt == T - 1),
                        )
                    # drip-feed half-0 drain into half-1's slack
                    if h == 1 and kt >= 1 and pending:
                        pending.pop(0)()
                if h == 0:
                    pending = post_half(0)
            for step in pending:
                step()
            for step in post_half(1):
                step()

    nc.compile()
    return nc


# ---------------------------------------------------------------------------
# General (non-trivial gamma/beta) fallback: original q-major design.
# ---------------------------------------------------------------------------

def build_kernel(L=2048, D=128, trivial_affine=True, qk_dt=F32R, att_dt=FP16):
    """Build + compile the single-core program (SPMD across 8 cores)."""
    P = 128
    ATT_DT = att_dt
    T = L // P                 # number of 128-row tiles
    CH = min(512, L)           # matmul moving-dim chunk
    NB = L // CH               # chunks per row
    TPB = max(1, min(4, T))    # q-tiles per PV block (512 q columns)
    scale = 1.0 / np.sqrt(np.float32(D))

    nc = bacc.Bacc(
        "TRN2",
        target_bir_lowering=False,
        debug=False,
        enable_asserts=False,
        num_devices=N_CORES,
    )
    q_d = nc.dram_tensor("q", [L, D], F32, kind="ExternalInput").ap()
    k_d = nc.dram_tensor("k", [L, D], F32, kind="ExternalInput").ap()
    v_d = nc.dram_tensor("v", [L, D], F32, kind="ExternalInput").ap()
    if not trivial_affine:
        g_d = nc.dram_tensor("gamma", [L], F32, kind="ExternalInput").ap()
        b_d = nc.dram_tensor("beta", [L], F32, kind="ExternalInput").ap()
    out_d = nc.dram_tensor("out", [L, D], F32, kind="ExternalOutput").ap()

    with tile.TileContext(nc) as tc, ExitStack() as ctx:
        consts = ctx.enter_context(tc.tile_pool(name="consts", bufs=1))
        small = ctx.enter_context(tc.tile_pool(name="small", bufs=4))

        # ---- persistent SBUF tensors -----------------------------------
        q_sb = consts.tile([P, T, D], F32)          # natural Q
        k_aug = consts.tile([P, T, D + 4], F32)     # natural K + ones col
        v_bf = consts.tile([P, T, D], ATT_DT)
        qT = consts.tile([P, L], F32)               # Q^T (d-major), fp32 for H
        qT_r = consts.tile([P, L], qk_dt)           # Q^T for QK matmul
        kT_r = consts.tile([P, L], qk_dt)           # K^T for QK matmul
        G_sb = consts.tile([P, D + 1], F32)         # K^T K | ksum
        a_all = consts.tile([P, T], F32)            # per-query exp scale
        b_all = consts.tile([P, T], F32)            # per-query exp bias
        rden_all = consts.tile([P, T], F32)         # 1/softmax-denominator
        ident_f = consts.tile([P, P], F32)
        ident_b = consts.tile([P, P], ATT_DT)
        eps_t = consts.tile([P, 1], F32)
        eps2_t = consts.tile([P, 1], F32)

        make_identity(nc, ident_f)
        make_identity(nc, ident_b)
        nc.vector.memset(eps_t, EPS)
        nc.vector.memset(eps2_t, EPS * D)
        nc.vector.memset(k_aug[:, :, D : D + 1], 1.0)

        if not trivial_affine:
            gam_bc = consts.tile([P, L], F32)
            bet_bc = consts.tile([P, L], F32)
            nc.sync.dma_start(out=gam_bc, in_=g_d.to_broadcast((P, L)))
            nc.sync.dma_start(out=bet_bc, in_=b_d.to_broadcast((P, L)))

        v_sb = consts.tile([P, T, D], F32)
        for t in range(T):
            r = slice(t * P, (t + 1) * P)
            nc.sync.dma_start(out=q_sb[:, t, :], in_=q_d[r, :])
            nc.sync.dma_start(out=k_aug[:, t, 0:D], in_=k_d[r, :])
            nc.sync.dma_start(out=v_sb[:, t, :], in_=v_d[r, :])
        nc.vector.tensor_copy(v_bf, v_sb)

        # ---- precompute: transposes, G, H, per-query stats -------------
        with tc.tile_pool(name="pre_ps", bufs=2, space="PSUM") as pre_ps:
            # Q^T / K^T via PE transpose
            for t in range(T):
                c = slice(t * P, (t + 1) * P)
                ps1 = pre_ps.tile([P, P], F32, tag="ps_tr")
                nc.tensor.transpose(ps1, q_sb[:, t, :], ident_f)
                nc.scalar.copy(out=qT[:, c], in_=ps1)
                nc.vector.tensor_copy(out=qT_r[:, c], in_=ps1)
                ps2 = pre_ps.tile([P, P], F32, tag="ps_tr")
                nc.tensor.transpose(ps2, k_aug[:, t, 0:D], ident_f)
                nc.scalar.copy(out=kT_r[:, c], in_=ps2)

            # G' = sum_c K_c^T [K_c | 1]  ->  [D, D+1]
            g_ps = pre_ps.tile([P, D + 1], F32, tag="ps_g")
            for t in range(T):
                nc.tensor.matmul(
                    g_ps,
                    lhsT=k_aug[:, t, 0:D],
                    rhs=k_aug[:, t, 0 : D + 1],
                    start=(t == 0),
                    stop=(t == T - 1),
                )
            nc.scalar.copy(out=G_sb, in_=g_ps)

            # Per-tile H' = Q_t G'  -> mean / var -> exp scale+bias
            for t in range(T):
                c = slice(t * P, (t + 1) * P)
                h_ps = pre_ps.tile([P, D + 1], F32, tag="ps_h")
                nc.tensor.matmul(
                    h_ps, lhsT=qT[:, c], rhs=G_sb, start=True, stop=True
                )
                h_sb = small.tile([P, D + 1], F32, tag="hsb")
                nc.scalar.copy(out=h_sb, in_=h_ps)
                nmu = small.tile([P, 1], F32, tag="nmu")
                junk = small.tile([P, D], F32, tag="junk")
                esq = small.tile([P, 1], F32, tag="esq")
                var = small.tile([P, 1], F32, tag="var")
                rstd = small.tile([P, 1], F32, tag="rstd")
                # nmu = -mean(S_t)  (scaled scores)
                nc.vector.tensor_scalar_mul(
                    nmu, h_sb[:, D : D + 1], -float(scale) / L
                )
                # esq = mean(S_t^2) = rowsum(H o Q) * scale^2 / L
                nc.vector.tensor_tensor(junk, h_sb[:, 0:D], q_sb[:, t, :], op=ALU.mult)
                nc.vector.tensor_reduce(esq, junk, axis=AX.X, op=ALU.add)
                nc.vector.tensor_scalar_mul(esq, esq, float(scale) * float(scale) / L)
                nc.vector.tensor_tensor(var, nmu, nmu, op=ALU.mult)
                nc.vector.tensor_sub(var, esq, var)
                nc.scalar.activation(rstd, var, AF.Sqrt, bias=eps_t)
                nc.vector.reciprocal(rstd, rstd)
                nc.vector.tensor_scalar_mul(
                    a_all[:, t : t + 1], rstd, float(scale)
                )
                nc.vector.tensor_tensor(
                    b_all[:, t : t + 1], nmu, rstd, op=ALU.mult
                )

        # ---- main loop: S -> exp -> transpose -> PV -> out -------------
        with (
            tc.tile_pool(name="s_ps", bufs=4, space="PSUM") as s_psp,
            tc.tile_pool(name="tr_ps", bufs=2, space="PSUM") as tr_psp,
            tc.tile_pool(name="o_ps", bufs=2, space="PSUM") as o_psp,
            tc.tile_pool(name="attn", bufs=3) as attn_pool,
            tc.tile_pool(name="attnT", bufs=2) as pT_pool,
            tc.tile_pool(name="osb", bufs=2) as osb_pool,
            tc.tile_pool(name="outp", bufs=3) as out_pool,
        ):
            n_blk = (T + TPB - 1) // TPB
            for j in range(n_blk):
                blkq = TPB * P  # q columns in this PV block
                attnT = pT_pool.tile([P, TPB, T, P], ATT_DT)
                for tt in range(TPB):
                    t = j * TPB + tt
                    qc = slice(t * P, (t + 1) * P)
                    attn = attn_pool.tile([P, L], ATT_DT)
                    dacc = small.tile([P, NB], F32, tag="dacc")
                    for nb in range(NB):
                        kc = slice(nb * CH, (nb + 1) * CH)
                        s_ps = s_psp.tile([P, CH], F32, tag="s")
                        nc.tensor.matmul(
                            s_ps,
                            lhsT=qT_r[:, qc],
                            rhs=kT_r[:, kc],
                            start=True,
                            stop=True,
                        )
                        if trivial_affine:
                            nc.scalar.activation(
                                attn[:, kc],
                                s_ps,
                                AF.Exp,
                                bias=b_all[:, t : t + 1],
                                scale=a_all[:, t : t + 1],
                                accum_out=dacc[:, nb : nb + 1],
                            )
                        else:
                            y = small.tile([P, CH], F32, tag="y")
                            nc.vector.tensor_scalar(
                                y,
                                in0=s_ps,
                                scalar1=a_all[:, t : t + 1],
                                scalar2=b_all[:, t : t + 1],
                                op0=ALU.mult,
                                op1=ALU.add,
                            )
                            nc.vector.tensor_tensor(
                                y, y, gam_bc[:, kc], op=ALU.mult
                            )
                            nc.vector.tensor_add(y, y, bet_bc[:, kc])
                            nc.scalar.activation(
                                attn[:, kc],
                                y,
                                AF.Exp,
                                accum_out=dacc[:, nb : nb + 1],
                            )
                    den = small.tile([P, 1], F32, tag="den")
                    nc.vector.tensor_reduce(
                        den, dacc, axis=AX.X, op=ALU.add
                    )
                    nc.vector.reciprocal(rden_all[:, t : t + 1], den)
                    # transpose attn row-block to key-major via DMA xbar
                    half = T // 2
                    for hh in range(2):
                        nc.sync.dma_start_transpose(
                            attnT[:, tt, hh * half : (hh + 1) * half, :],
                            attn[:, hh * half * P : (hh + 1) * half * P],
                        )
                # PV for this block: out^T accumulation over key chunks
                o_ps = o_psp.tile([P, blkq], F32, tag="o")
                for ct in range(T):
                    nc.tensor.matmul(
                        o_ps,
                        lhsT=v_bf[:, ct, :],
                        rhs=attnT[:, :, ct, :],
                        start=(ct == 0),
                        stop=(ct == T - 1),
                    )
                oT = osb_pool.tile([P, blkq], F32)
                nc.scalar.copy(out=oT, in_=o_ps)
                for tt in range(TPB):
                    t = j * TPB + tt
                    tr2 = tr_psp.tile([P, P], F32, tag="tr")
                    nc.tensor.transpose(
                        tr2, oT[:, tt * P : (tt + 1) * P], ident_f
                    )
                    ot = out_pool.tile([P, D], F32)
                    nc.scalar.activation(
                        ot, tr2, AF.Copy, scale=rden_all[:, t : t + 1]
                    )
                    nc.sync.dma_start(
                        out=out_d[t * P : (t + 1) * P, :], in_=ot
                    )

    nc.compile()
    return nc


_CACHE = {}


def _get_nc(L, D, trivial):
    key = (L, D, trivial)
    if key not in _CACHE:
        if trivial:
            _CACHE[key] = build_kernel_fast(L, D)
        else:
            _CACHE[key] = build_kernel(L, D, trivial_affine=False)
    return _CACHE[key]


def kernel(q, k, v, gamma, beta, _trace=False):
    n, L, D = q.shape
    assert n == N_CORES
    trivial = bool(np.all(gamma == 1.0) and np.all(beta == 0.0))
    nc = _get_nc(L, D, trivial)
    in_maps = []
    for c in range(n):
        m = {
            "q": np.ascontiguousarray(q[c], dtype=np.float32),
            "k": np.ascontiguousarray(k[c], dtype=np.float32),
            "v": np.ascontiguousarray(v[c], dtype=np.float32),
        }
        if not trivial:
            m["gamma"] = np.ascontiguousarray(gamma, dtype=np.float32)
            m["beta"] = np.ascontiguousarray(beta, dtype=np.float32)
        in_maps.append(m)
    res = bass_utils.run_bass_kernel_spmd(
        nc, in_maps, core_ids=list(range(n)), trace=_trace
    )
    out = np.stack([res.results[c]["out"] for c in range(n)], axis=0)
    if _trace:
        kernel.last_exec_time_ns = res.exec_time_ns
        kernel.last_results = res
    return out.astype(np.float32)
